# revision 16
# baseline (speedup 1.0000x reference)
"""ChessGNN (2-layer GCN + global max pool + FC + log_softmax) on 8 Trainium2 cores.

Strategy (edge-parallel, dst-range sharded, hardware-looped):
  - Core k owns dst nodes [12500k, 12500(k+1)). Edges are routed to the core
    owning their dst, ordered bucket-major: bucket = 128 consecutive dst
    nodes, and within a bucket by src-range (int16 gather windows of 32768
    padded-global rows). Every (bucket, range) cell is padded to a uniform
    chunk count nu[range], so the per-bucket work is identical and the whole
    aggregation runs as a single For_i hardware loop (tiny program).
  - deg/dinv = 1/sqrt(1+deg) is computed on the HOST (data-independent of x)
    and shipped per-core as [128, NB] f32 — no on-device degree pass.
  - Per layer: hs = (h_prev @ W) * dinv for the node slice, AllGather ->
    [100352, 64] f32 table in DRAM; per bucket: gpsimd dma_gather fetches the
    edges' source rows, a one-hot [128e, 128slot] (DVE is_equal vs iota)
    matmul-accumulates the segment sum in PSUM over all V chunks.
  - Gather idx slabs ship as [16, cols] int16 and are replicated to 128
    partitions on-device (8 DMAs); dst-local offsets ship as uint8 (255 pad)
    and are cast to f32 on-device.
  - Head: strided max-reduce over buckets -> PE transpose -> AllReduce(max)
    -> FC -> log_softmax.
"""
import os

import numpy as np

import jax

# Persistent XLA/PJRT compilation cache: the program below is deterministic
# for given inputs, so warm runs skip the backend compile entirely.
try:
    jax.config.update("jax_compilation_cache_dir",
                      os.path.expanduser("~/.cache/jax_bass"))
    jax.config.update("jax_persistent_cache_min_entry_size_bytes", -1)
    jax.config.update("jax_persistent_cache_min_compile_time_secs", 0)
except Exception:
    pass

import concourse.bass as bass
import concourse.bacc as bacc
import concourse.mybir as mybir
import concourse.tile as tile
from concourse.bass import ds
from concourse.bass_utils import run_bass_kernel_spmd
from concourse.masks import make_identity
from ml_dtypes import bfloat16

N = 100000
NCORES = 8
S = N // NCORES            # 12500 nodes per core
NB = 98                    # buckets of 128 dst nodes (98*128 = 12544)
SP = NB * 128              # padded slice rows
NPAD = NCORES * SP         # padded global rows = 100352
RNG = 32768                # int16 gather range
NRANGES = (NPAD + RNG - 1) // RNG  # 4

LAST_RESULTS = None
RUN_WALL_NS = None


def kernel(x, edge_index, W1, b1, W2, b2, fcW, fcb):
    global LAST_RESULTS, RUN_WALL_NS
    x = np.asarray(x, np.float32)
    ei = np.asarray(edge_index)
    src = ei[0].astype(np.int64)
    dst = ei[1].astype(np.int64)

    # ---- host: degree (data-independent of features); dinv computed on
    # device as 1/sqrt(deg+1). Padded slots get deg 0 -> dinv 1 (harmless:
    # their h_prev rows are zero and no edge targets them).
    deg16 = np.bincount(dst, minlength=N)
    assert deg16.max() < 65536
    deg16 = deg16.astype(np.uint16)

    owner = dst // S
    src_gid = (src // S) * SP + (src % S)      # padded global row id of src
    g_r = src_gid >> 15                        # gather range
    dl = dst - owner * S                       # dst local id
    b_b = dl >> 7                              # bucket
    slot = dl & 127                            # slot within bucket

    # uniform chunks per (bucket, range): nu[g] = max over (core,bucket)
    key = (owner * NB + b_b) * NRANGES + g_r
    cnt = np.bincount(key, minlength=NCORES * NB * NRANGES) \
        .reshape(NCORES, NB, NRANGES)
    nu = np.maximum(1, (cnt.max(axis=(0, 1)) + 127) // 128).astype(np.int64)
    V = int(nu.sum())                          # chunks per bucket
    offV = np.zeros(NRANGES, np.int64)
    offV[1:] = np.cumsum(nu)[:-1]
    off8 = offV * 8
    CB = 8 * V                                 # g16 cols per bucket

    g16_all, dl8_all, dv_all, xT_all = [], [], [], []
    for k in range(NCORES):
        m = owner == k
        sgk, gk, bk, slk = src_gid[m], g_r[m], b_b[m], slot[m]
        order = np.lexsort((gk, bk))
        sgk, gk, bk, slk = sgk[order], gk[order], bk[order], slk[order]
        Ek = len(sgk)
        gid = bk * NRANGES + gk
        bound = np.r_[True, gid[1:] != gid[:-1]]
        starts = np.flatnonzero(bound)
        seg = np.cumsum(bound) - 1
        r = np.arange(Ek) - starts[seg]        # rank within (bucket, range)

        g16 = np.zeros((16, NB * CB), np.int16)         # pad idx 0 (safe row)
        g16[r % 16, bk * CB + off8[gk] + r // 16] = (sgk - gk * RNG).astype(np.int16)
        dl8 = np.full((128, NB * V), 255, np.uint8)     # 255 = pad (no slot)
        dl8[r % 128, bk * V + offV[gk] + r // 128] = slk.astype(np.uint8)
        g16_all.append(np.ascontiguousarray(g16))
        dl8_all.append(np.ascontiguousarray(dl8))

        dgk = np.zeros(SP, np.uint16)
        dgk[:S] = deg16[k * S:(k + 1) * S]
        dv_all.append(np.ascontiguousarray(dgk.reshape(NB, 128).T))

        xs = np.zeros((SP, 8), np.float32)
        xs[:S] = x[k * S:(k + 1) * S]
        xT_all.append(np.ascontiguousarray(xs.T.astype(bfloat16)))

    b1r = np.asarray(b1, np.float32)[None, :]
    b2r = np.asarray(b2, np.float32)[None, :]
    fcb2 = np.asarray(fcb, np.float32)[None, :]

    # ---- build the SPMD program ----
    nc = bacc.Bacc("TRN2", target_bir_lowering=False, debug=False, num_devices=NCORES)
    dt = mybir.dt
    xT_t = nc.dram_tensor("xT", [8, SP], dt.bfloat16, kind="ExternalInput")
    g16_t = nc.dram_tensor("g16", [16, NB * CB], dt.int16, kind="ExternalInput")
    dl8_t = nc.dram_tensor("dl8", [128, NB * V], dt.uint8, kind="ExternalInput")
    dv_t = nc.dram_tensor("dv", [128, NB], dt.uint16, kind="ExternalInput")
    W1_t = nc.dram_tensor("W1", [8, 32], dt.bfloat16, kind="ExternalInput")
    W2_t = nc.dram_tensor("W2", [32, 32], dt.float32, kind="ExternalInput")
    b1_t = nc.dram_tensor("b1t", [1, 32], dt.float32, kind="ExternalInput")
    b2_t = nc.dram_tensor("b2t", [1, 32], dt.float32, kind="ExternalInput")
    fcW_t = nc.dram_tensor("fcW", [32, 5], dt.float32, kind="ExternalInput")
    fcb_t = nc.dram_tensor("fcb", [1, 5], dt.float32, kind="ExternalInput")
    out_t = nc.dram_tensor("out", [1, 5], dt.float32, kind="ExternalOutput")

    AF = mybir.ActivationFunctionType
    ALU = mybir.AluOpType
    AX = mybir.AxisListType

    with tile.TileContext(nc) as tc:
        with (
            tc.tile_pool(name="per", bufs=1) as per_p,
            tc.tile_pool(name="st", bufs=2) as st_p,
            tc.tile_pool(name="gt", bufs=2) as gt_p,
            tc.tile_pool(name="oh", bufs=2) as oh_p,
            tc.tile_pool(name="ps", bufs=2, space="PSUM") as ps_p,
            tc.tile_pool(name="psd", bufs=1, space="PSUM") as psd_p,
            tc.tile_pool(name="dram", bufs=1, space="DRAM") as dram_p,
        ):
            XT = per_p.tile([8, SP], dt.bfloat16)
            G16 = per_p.tile([128, NB * CB], dt.int16)
            DL8 = per_p.tile([128, NB * V], dt.uint8)
            DLF = per_p.tile([128, NB * V], dt.float32)
            DEGF = per_p.tile([128, NB], dt.uint16)
            DINV = per_p.tile([128, NB], dt.float32)
            IO = per_p.tile([128, 128], dt.float32)
            IDN = per_p.tile([128, 128], dt.float32)
            W1s = per_p.tile([8, 32], dt.bfloat16)
            W2s = per_p.tile([32, 32], dt.float32)
            B1IN = per_p.tile([1, 32], dt.float32)
            B2IN = per_p.tile([1, 32], dt.float32)
            B1 = per_p.tile([128, 32], dt.float32)
            B2 = per_p.tile([128, 32], dt.float32)
            ONE1 = per_p.tile([1, 128], dt.float32)
            FCW = per_p.tile([32, 5], dt.float32)
            FCB = per_p.tile([1, 5], dt.float32)
            P = per_p.tile([128, NB * 32], dt.float32)   # h_prev @ W (slice)
            H = per_p.tile([128, NB * 32], dt.float32)   # layer output
            HS = per_p.tile([128, NB, 32], dt.float32)   # P * dinv
            ACC = per_p.tile([128, NB * 32], dt.float32) # aggregated messages
            T1 = per_p.tile([128, NB, 32], dt.float32)

            nc.sync.dma_start(XT[:], xT_t[:, :])
            for kk in range(8):
                nc.sync.dma_start(G16[16 * kk:16 * kk + 16, :], g16_t[:, :])
            for t_, s_ in ((DL8, dl8_t), (DEGF, dv_t), (W1s, W1_t), (W2s, W2_t),
                           (B1IN, b1_t), (B2IN, b2_t), (FCW, fcW_t), (FCB, fcb_t)):
                nc.sync.dma_start(t_[:], s_[:, :])
            nc.vector.tensor_copy(DLF[:], DL8[:])
            # dinv = 1/sqrt(deg + 1)
            SQ = per_p.tile([128, NB], dt.float32)
            nc.scalar.activation(SQ[:], DEGF[:], AF.Sqrt, bias=1.0)
            nc.vector.reciprocal(DINV[:], SQ[:])
            nc.gpsimd.iota(IO[:], pattern=[[1, 128]], base=0,
                           channel_multiplier=0,
                           allow_small_or_imprecise_dtypes=True)
            make_identity(nc, IDN[:])
            # partition-broadcast biases via PE: B = ones[128,1] @ b[1,32]
            nc.vector.memset(ONE1[:], 1.0)
            for bin_, bt_ in ((B1IN, B1), (B2IN, B2)):
                psb = psd_p.tile([128, 32], dt.float32, tag="pbc")
                nc.tensor.matmul(psb[:], lhsT=ONE1[:], rhs=bin_[:],
                                 start=True, stop=True)
                nc.scalar.copy(bt_[:], psb[:])

            agin1 = dram_p.tile([SP, 64], dt.float32)
            agout1 = dram_p.tile([NPAD, 64], dt.float32)
            agin2 = dram_p.tile([SP, 64], dt.float32)
            agout2 = dram_p.tile([NPAD, 64], dt.float32)
            arin = dram_p.tile([32, 1], dt.float32)
            arout = dram_p.tile([32, 1], dt.float32)

            P3 = P[:].rearrange("p (b f) -> p b f", f=32)
            H3 = H[:].rearrange("p (b f) -> p b f", f=32)
            ACC3 = ACC[:].rearrange("p (b f) -> p b f", f=32)
            dinv_b = DINV[:].rearrange("p (b o) -> p b o", o=1) \
                .to_broadcast([128, NB, 32])

            # ---- P = x @ W1 ----
            with tc.For_i(0, NB, 2) as t:
                for u in range(2):
                    xtile = st_p.tile([8, 128], dt.bfloat16, tag="xtile")
                    nc.sync.dma_start(xtile[:], XT[:, ds(t * 128 + u * 128, 128)])
                    psm = ps_p.tile([128, 32], dt.float32, tag="pmm")
                    nc.tensor.matmul(psm[:], lhsT=xtile[:], rhs=W1s[:],
                                     start=True, stop=True)
                    pst = st_p.tile([128, 32], dt.float32, tag="pst")
                    nc.scalar.copy(pst[:], psm[:])
                    nc.sync.dma_start(P[:, ds(t * 32 + u * 32, 32)], pst[:])

            def aggregate(agout):
                """per bucket: gather 4 src ranges + one-hot matmul segment sum."""
                with tc.For_i(0, NB, 2) as i:
                    for u in range(2):
                        slab = st_p.tile([128, CB], dt.int16, tag="slab")
                        nc.sync.dma_start(slab[:], G16[:, ds(i * CB + u * CB, CB)])
                        dlfb = st_p.tile([128, V], dt.float32, tag="dlfb")
                        nc.sync.dma_start(dlfb[:], DLF[:, ds(i * V + u * V, V)])
                        ohh = oh_p.tile([128, V, 128], dt.float32, tag="oh")
                        nc.vector.tensor_tensor(
                            out=ohh[:],
                            in0=dlfb[:].rearrange("p (c o) -> p c o", o=1)
                                .to_broadcast([128, V, 128]),
                            in1=IO[:].rearrange("p (o s) -> p o s", o=1)
                                .to_broadcast([128, V, 128]),
                            op=ALU.is_equal)
                        gts = []
                        for g in range(NRANGES):
                            r0, r1 = g * RNG, min((g + 1) * RNG, NPAD)
                            nug = int(nu[g])
                            gtg = gt_p.tile([128, nug, 64], dt.float32, tag=f"gt{g}")
                            nc.gpsimd.dma_gather(
                                gtg[:], agout[r0:r1, :],
                                slab[:, int(off8[g]):int(off8[g]) + nug * 8],
                                nug * 128, nug * 128, 64)
                            gts.append(gtg)
                        psm = ps_p.tile([128, 32], dt.float32, tag="pagg")
                        cpos = 0
                        for g in range(NRANGES):
                            for c in range(int(nu[g])):
                                nc.tensor.matmul(psm[:], lhsT=ohh[:, cpos, :],
                                                 rhs=gts[g][:, c, 0:32],
                                                 start=(cpos == 0),
                                                 stop=(cpos == V - 1))
                                cpos += 1
                        accst = st_p.tile([128, 32], dt.float32, tag="accst")
                        nc.scalar.copy(accst[:], psm[:])
                        nc.sync.dma_start(ACC[:, ds(i * 32 + u * 32, 32)], accst[:])

            def combine(Bt):
                """H = relu(dinv*(ACC + dinv*P) + b)"""
                nc.vector.tensor_mul(T1[:], P3, dinv_b)
                nc.vector.tensor_add(T1[:], T1[:], ACC3)
                nc.vector.tensor_mul(T1[:], T1[:], dinv_b)
                nc.vector.tensor_add(
                    T1[:], T1[:],
                    Bt[:].rearrange("p (o f) -> p o f", o=1)
                        .to_broadcast([128, NB, 32]))
                nc.scalar.activation(H3, T1[:], AF.Relu)

            # ---- layer 1 ----
            nc.vector.tensor_mul(HS[:], P3, dinv_b)
            nc.sync.dma_start(
                agin1[:, :].rearrange("(a p) b -> p a b", p=128)[:, :, 0:32], HS[:])
            nc.gpsimd.collective_compute(
                "AllGather", ALU.bypass, replica_groups=[list(range(NCORES))],
                ins=[agin1.opt()], outs=[agout1.opt()])
            aggregate(agout1)
            combine(B1)

            # ---- P = h1 @ W2 (per-tile transpose) ----
            with tc.For_i(0, NB, 2) as t:
                for u in range(2):
                    htile = st_p.tile([128, 32], dt.float32, tag="htile")
                    nc.sync.dma_start(htile[:], H[:, ds(t * 32 + u * 32, 32)])
                    pstr = psd_p.tile([32, 128], dt.float32, tag="ptr")
                    nc.tensor.transpose(out=pstr[:], in_=htile[:], identity=IDN[:])
                    h1t = st_p.tile([32, 128], dt.float32, tag="h1t")
                    nc.scalar.copy(h1t[:], pstr[:])
                    psm = ps_p.tile([128, 32], dt.float32, tag="pmm")
                    nc.tensor.matmul(psm[:], lhsT=h1t[:], rhs=W2s[:],
                                     start=True, stop=True)
                    pst = st_p.tile([128, 32], dt.float32, tag="pst2")
                    nc.scalar.copy(pst[:], psm[:])
                    nc.sync.dma_start(P[:, ds(t * 32 + u * 32, 32)], pst[:])

            # ---- layer 2 ----
            nc.vector.tensor_mul(HS[:], P3, dinv_b)
            nc.sync.dma_start(
                agin2[:, :].rearrange("(a p) b -> p a b", p=128)[:, :, 0:32], HS[:])
            nc.gpsimd.collective_compute(
                "AllGather", ALU.bypass, replica_groups=[list(range(NCORES))],
                ins=[agin2.opt()], outs=[agout2.opt()])
            aggregate(agout2)
            combine(B2)

            # ---- head: global max pool + FC + log_softmax ----
            GMAX = per_p.tile([128, 32], dt.float32)
            nc.vector.reduce_max(
                GMAX[:].rearrange("p (f o) -> p f o", o=1),
                H[:].rearrange("p (b f) -> p f b", f=32), axis=AX.X)
            psg = psd_p.tile([32, 128], dt.float32, tag="ptr")
            nc.tensor.transpose(out=psg[:], in_=GMAX[:], identity=IDN[:])
            GT = per_p.tile([32, 128], dt.float32)
            nc.scalar.copy(GT[:], psg[:])
            GV = per_p.tile([32, 1], dt.float32)
            nc.vector.reduce_max(GV[:], GT[:], axis=AX.X)
            nc.sync.dma_start(arin[:, :], GV[:])
            nc.gpsimd.collective_compute(
                "AllReduce", ALU.max, replica_groups=[list(range(NCORES))],
                ins=[arin.opt()], outs=[arout.opt()])
            GAR = per_p.tile([32, 1], dt.float32)
            nc.sync.dma_start(GAR[:], arout[:, :])
            psl = psd_p.tile([1, 5], dt.float32, tag="plg")
            nc.tensor.matmul(psl[:], lhsT=GAR[:], rhs=FCW[:], start=True, stop=True)
            LG = per_p.tile([1, 5], dt.float32)
            nc.vector.tensor_add(LG[:], psl[:], FCB[:])
            MX = per_p.tile([1, 1], dt.float32)
            nc.vector.reduce_max(MX[:], LG[:], axis=AX.X)
            nc.vector.tensor_tensor(LG[:], LG[:], MX[:].to_broadcast([1, 5]),
                                    op=ALU.subtract)
            EX = per_p.tile([1, 5], dt.float32)
            nc.scalar.activation(EX[:], LG[:], AF.Exp)
            SM = per_p.tile([1, 1], dt.float32)
            nc.vector.reduce_sum(SM[:], EX[:], axis=AX.X)
            LS = per_p.tile([1, 1], dt.float32)
            nc.scalar.activation(LS[:], SM[:], AF.Ln)
            nc.vector.tensor_tensor(LG[:], LG[:], LS[:].to_broadcast([1, 5]),
                                    op=ALU.subtract)
            nc.sync.dma_start(out_t[:, :], LG[:])

    nc.compile()

    in_maps = []
    for k in range(NCORES):
        in_maps.append({
            "xT": xT_all[k], "g16": g16_all[k], "dl8": dl8_all[k],
            "dv": dv_all[k],
            "W1": np.asarray(W1, np.float32).astype(bfloat16),
            "W2": np.asarray(W2, np.float32),
            "b1t": b1r, "b2t": b2r, "fcW": np.asarray(fcW, np.float32),
            "fcb": fcb2,
        })
    import os, time as _time
    trace = os.environ.get("GNN_TRACE", "0") == "1"
    _t0 = _time.time()
    res = run_bass_kernel_spmd(nc, in_maps, core_ids=list(range(NCORES)), trace=trace)
    RUN_WALL_NS = int((_time.time() - _t0) * 1e9)
    LAST_RESULTS = res
    return res.results[0]["out"].astype(np.float32)
